# revision 24
# baseline (speedup 1.0000x reference)
"""GINEConv + 2-layer MLP + residual + BatchNorm on 8 Trainium2 NeuronCores.

v2 strategy (graph/data parallel, per sharding hint):
- Partition dst nodes contiguously across 8 cores (npc=6272/core). Each core
  owns the edges incident to its dst nodes.
- x is shipped as per-core bf16 shards and AllGathered on device (12.8 MB of
  H2D instead of 8x replicated fp32 x = 205 MB).
- Edge gather uses batched SWDGE dma_gather (int16 indices, one call per
  ~64 subtiles) instead of one 128-descriptor indirect DMA per subtile.
  int16 index range forces a 2-halves split of x; edges are processed in two
  passes (src < HALF, src >= HALF) with pass-A aggregates staged in SBUF.
- edge_attr is shipped fp16, host-permuted to [128, nsub, 128] so each chunk
  load is one contiguous-per-partition HWDGE DMA.
- Messages msg = relu(xg + ea) in bf16; scatter-add into a 128-node dst
  window via one-hot matmul (S built on DVE by iota/is_equal, pad edges get
  rel=-1 so their mask column is all-zero and they never land).
- Per window: h = x + aggr; h2 = x + (relu(h@W1+b1)@W2+b2) with bf16 weights
  stationary; BN partial sums accumulated on the fly (fp32).
- BN: one AllReduce of [128, 2] (sum, sumsq), biased variance, then a
  normalize + PE-transpose + store pass (fp32).

kernel(**inputs) takes FULL inputs, returns FULL [50000, 128] output.
"""
import numpy as np
import ml_dtypes

import concourse.bass as bass
import concourse.mybir as mybir
import concourse.tile as tile
import concourse.bacc as bacc
import concourse.bass_utils as bass_utils
from concourse import library_config
from concourse.masks import make_identity

P = 128
D = 128
NCORES = 8
BN_EPS = 1e-5
CH = 32          # subtiles per gather/ea chunk (32*128 = 4096 edges)

F32 = mybir.dt.float32
BF16 = mybir.dt.float16
I16 = mybir.dt.int16
BF = np.float16


# ----------------------------------------------------------------- host prep
def _prep(x, edge_index, edge_attr):
    """Partition + pad edges; build per-core arrays (identical shapes)."""
    N = x.shape[0]
    npc = ((N + NCORES - 1) // NCORES + P - 1) // P * P     # 6272
    nw = npc // P                                            # 49
    ntot = NCORES * npc                                      # 50176
    half = ((N + 1) // 2 + P - 1) // P * P                   # 25088
    assert half < 32768 and ntot - half < 32768

    src = edge_index[0].astype(np.int64)
    dst = edge_index[1].astype(np.int64)
    core = np.minimum(dst // npc, NCORES - 1)
    ldst = dst - core * npc
    win = ldst // P
    rel = ldst % P
    hlf = (src >= half).astype(np.int64)

    # counts per (core, half, window) -> shared subtile counts (max over cores)
    counts = np.zeros((NCORES, 2, nw), np.int64)
    np.add.at(counts, (core, hlf, win), 1)
    tw = np.maximum(1, (counts.max(axis=0) + P - 1) // P)    # [2, nw]
    nsubA, nsubB = int(tw[0].sum()), int(tw[1].sum())
    nsub = nsubA + nsubB
    epad = nsub * P

    # slot base for each (half, window)
    base = np.zeros((2, nw), np.int64)
    flat_tw = np.concatenate([tw[0], tw[1]])
    starts = np.concatenate([[0], (flat_tw * P).cumsum()])[:-1]
    base[0] = starts[:nw]
    base[1] = starts[nw:]

    # order edges per (core, half, window); assign slot ids
    order = np.lexsort((win, hlf, core))
    core_o, hlf_o, win_o = core[order], hlf[order], win[order]
    # inner rank within each (core, half, window) group
    grp = (core_o * 2 + hlf_o) * nw + win_o
    gcounts = np.bincount(grp, minlength=NCORES * 2 * nw)
    gstart = np.concatenate([[0], gcounts.cumsum()])[:-1]
    inner = np.arange(len(order)) - np.repeat(gstart, gcounts)
    slot = base[hlf_o, win_o] + inner

    ea_dev = np.zeros((NCORES, P, nsub, P), BF)
    idx_dev = np.zeros((NCORES, 16, nsub * 8), np.int16)
    rel_dev = np.full((NCORES, P, nsub), -1.0, np.float32)
    eab = edge_attr.astype(BF)
    for c in range(NCORES):
        m = core_o == c
        eids = order[m]
        sl = slot[m]
        s_sub, s_lane = sl // P, sl % P
        ea_dev[c, s_lane, s_sub, :] = eab[eids]
        rel_dev[c, s_lane, s_sub] = rel[eids].astype(np.float32)
        # gather indices, local to half, wrapped [i%16, i//16] per half
        lidx = (src[eids] - hlf_o[m] * half).astype(np.int16)
        gi = sl.copy()
        gi[hlf_o[m] == 1] -= nsubA * P       # position within half
        col0 = np.where(hlf_o[m] == 1, nsubA * 8, 0)
        idx_dev[c, gi % 16, col0 + gi // 16] = lidx

    # x shards (bf16, padded rows zero)
    xsh = np.zeros((NCORES, npc, D), BF)
    for c in range(NCORES):
        lo, hi = c * npc, min(N, (c + 1) * npc)
        xsh[c, :hi - lo] = x[lo:hi].astype(BF)

    npad_nodes = np.zeros((NCORES, P), np.float32)
    npad_nodes[NCORES - 1, :] = ntot - N
    return dict(nw=nw, tw=tw, nsubA=nsubA, nsubB=nsubB, nsub=nsub,
                epad=epad, npc=npc, half=half, ntot=ntot,
                ea_dev=ea_dev.reshape(NCORES, P, nsub * P),
                idx_dev=idx_dev, rel_dev=rel_dev, xsh=xsh, npad=npad_nodes)


# ------------------------------------------------------------- device program
def build_nc(nw, tw, nsubA, nsubB, npc, half, ntot, N, repeat=1, ablate="",
             only=""):
    abl = set(ablate.split(",")) if ablate else set()
    onl = set(only.split(",")) if only else set()
    nsub = nsubA + nsubB
    nc = bacc.Bacc("TRN2", target_bir_lowering=False, debug=False,
                   num_devices=NCORES)
    t_xsh = nc.dram_tensor("xsh", [npc, D], BF16, kind="ExternalInput").ap()
    t_ea = nc.dram_tensor("ea", [P, nsub * P], BF16, kind="ExternalInput").ap()
    t_idx = nc.dram_tensor("idx", [16, nsub * 8], I16, kind="ExternalInput").ap()
    t_rel = nc.dram_tensor("rel", [P, nsub], F32, kind="ExternalInput").ap()
    t_w1 = nc.dram_tensor("W1", [D, D], F32, kind="ExternalInput").ap()
    t_w2 = nc.dram_tensor("W2", [D, D], F32, kind="ExternalInput").ap()
    t_b1 = nc.dram_tensor("b1", [D], F32, kind="ExternalInput").ap()
    t_b2 = nc.dram_tensor("b2", [D], F32, kind="ExternalInput").ap()
    t_bnw = nc.dram_tensor("bn_w", [D], F32, kind="ExternalInput").ap()
    t_bnb = nc.dram_tensor("bn_b", [D], F32, kind="ExternalInput").ap()
    t_npad = nc.dram_tensor("npad", [P], F32, kind="ExternalInput").ap()
    t_out = nc.dram_tensor("out", [npc, D], F32, kind="ExternalOutput").ap()

    # per-(half, window) subtile spans
    spans = []                   # (half, w, start_subtile_in_half, count)
    for h in range(2):
        s = 0
        for w in range(nw):
            spans.append((h, w, s, int(tw[h][w])))
            s += int(tw[h][w])

    with tile.TileContext(nc) as tc:
        with (
            tc.tile_pool(name="const", bufs=1) as cpool,
            tc.tile_pool(name="iox", bufs=4) as iox,
            tc.tile_pool(name="ioe", bufs=4) as ioe,
            tc.tile_pool(name="work", bufs=10) as work,
            tc.tile_pool(name="mwork", bufs=10) as mwork,
            tc.tile_pool(name="psA", bufs=2, space="PSUM") as psA,
            tc.tile_pool(name="psB", bufs=2, space="PSUM") as psB,
            tc.tile_pool(name="psC", bufs=2, space="PSUM") as psC,
            tc.tile_pool(name="psD", bufs=2, space="PSUM") as psD,
            tc.tile_pool(name="psE", bufs=2, space="PSUM") as psE,
            tc.tile_pool(name="dram", bufs=1, space="DRAM") as dram,
        ):
            # ---- constants
            w1_sb = cpool.tile([P, D], BF16)
            nc.gpsimd.dma_start(out=w1_sb[:], in_=t_w1[:])   # cast fp32->bf16
            w2_sb = cpool.tile([P, D], BF16)
            nc.gpsimd.dma_start(out=w2_sb[:], in_=t_w2[:])
            b1_sb = cpool.tile([P, 1], F32)
            nc.sync.dma_start(out=b1_sb[:], in_=t_b1[:, None])
            b2_sb = cpool.tile([P, 1], F32)
            nc.sync.dma_start(out=b2_sb[:], in_=t_b2[:, None])
            bnw_sb = cpool.tile([P, 1], F32)
            nc.sync.dma_start(out=bnw_sb[:], in_=t_bnw[:, None])
            bnb_sb = cpool.tile([P, 1], F32)
            nc.sync.dma_start(out=bnb_sb[:], in_=t_bnb[:, None])
            npad_sb = cpool.tile([P, 1], F32)
            nc.sync.dma_start(out=npad_sb[:], in_=t_npad[:, None])
            idx_sb = cpool.tile([P, nsub * 8], I16)
            for k in range(8):
                nc.sync.dma_start(out=idx_sb[16 * k:16 * (k + 1), :],
                                  in_=t_idx[:])
            rel_sb = cpool.tile([P, nsub], F32)
            nc.sync.dma_start(out=rel_sb[:], in_=t_rel[:])
            iota_i = cpool.tile([P, P], mybir.dt.int32)
            nc.gpsimd.iota(iota_i[:], pattern=[[1, P]], base=0,
                           channel_multiplier=0)
            iota_bf = cpool.tile([P, P], BF16)
            nc.vector.tensor_copy(out=iota_bf[:], in_=iota_i[:])
            ident = cpool.tile([P, P], F32)
            make_identity(nc, ident[:])

            # xt = x_local^T via HWDGE DMA transpose (bf16)
            xt_sb = cpool.tile([P, npc], BF16)
            nc.sync.dma_start_transpose(out=xt_sb[:], in_=t_xsh[:])

            # AllGather x shards -> full x in HBM
            xin = dram.tile([npc, D], BF16)
            nc.sync.dma_start(out=xin[:], in_=t_xsh[:])
            xfull = dram.tile([ntot, D], BF16, addr_space="Shared")
            nc.gpsimd.collective_compute(
                "AllGather", mybir.AluOpType.bypass,
                replica_groups=[list(range(NCORES))],
                ins=[xin.opt()], outs=[xfull.opt()])

            aggrA_sb = cpool.tile([P, npc], BF16)
            h2_sb = cpool.tile([P, npc], F32)
            sums = cpool.tile([P, nw], F32)
            sumsq = cpool.tile([P, nw], F32)

            def emit_main():
                for h in range(2):
                    nsub_h = nsubA if h == 0 else nsubB
                    col0 = 0 if h == 0 else nsubA * 8
                    sub0 = 0 if h == 0 else nsubA
                    src_lo = 0 if h == 0 else half
                    src_hi = half if h == 0 else ntot
                    # window lookup for this half
                    w_of, t_of, last_of = [], [], []
                    for (hh, w, s, cnt) in spans:
                        if hh != h:
                            continue
                        for t in range(cnt):
                            w_of.append(w)
                            t_of.append(t)
                            last_of.append(t == cnt - 1)
                    aggr_ps = None
                    for cs in range(0, nsub_h, CH):
                        clen = min(CH, nsub_h - cs)
                        xg = iox.tile([P, clen * P], BF16, tag="xg")
                        nc.gpsimd.dma_gather(
                            xg[:].rearrange("p (s f) -> p s f", f=P),
                            xfull[src_lo:src_hi, :],
                            idx_sb[:, col0 + cs * 8: col0 + (cs + clen) * 8],
                            clen * P, clen * P, P, single_packet=False)
                        if "gather" in abl:
                            xg2 = iox.tile([P, clen * P], BF16, tag="xg2", bufs=1)
                            nc.gpsimd.dma_gather(
                                xg2[:].rearrange("p (s f) -> p s f", f=P),
                                xfull[src_lo:src_hi, :],
                                idx_sb[:, col0 + cs * 8:
                                       col0 + (cs + clen) * 8],
                                clen * P, clen * P, P, single_packet=False)
                        ea_t = None
                        if not onl or "ea" in onl:
                            ea_t = ioe.tile([P, clen * P], BF16, tag="ea")
                            nc.sync.dma_start(
                                out=ea_t[:],
                                in_=t_ea[:, (sub0 + cs) * P:
                                         (sub0 + cs + clen) * P])
                        if "ea" in abl:
                            ea2 = ioe.tile([P, clen * P], BF16, tag="ea2", bufs=1)
                            nc.sync.dma_start(
                                out=ea2[:],
                                in_=t_ea[:, (sub0 + cs) * P:
                                         (sub0 + cs + clen) * P])
                        msgs = {}
                        do_msg = (not onl) or "msg" in onl
                        do_scatter = not onl
                        for s4 in (range(0, clen, 4) if do_msg else []):
                            sl = min(4, clen - s4)
                            sum_t = work.tile([P, sl * P], BF16, tag="sum", bufs=4)
                            nc.vector.tensor_add(
                                out=sum_t[:], in0=xg[:, s4 * P:(s4 + sl) * P],
                                in1=ea_t[:, s4 * P:(s4 + sl) * P])
                            if "add" in abl:
                                sum2 = work.tile([P, sl * P], BF16, tag="sum2", bufs=2)
                                nc.vector.tensor_add(
                                    out=sum2[:],
                                    in0=xg[:, s4 * P:(s4 + sl) * P],
                                    in1=ea_t[:, s4 * P:(s4 + sl) * P])
                            msg_t = work.tile([P, sl * P], BF16, tag="msg", bufs=6)
                            nc.scalar.activation(
                                out=msg_t[:], in_=sum_t[:],
                                func=mybir.ActivationFunctionType.Relu)
                            if "relu" in abl:
                                msg2 = work.tile([P, sl * P], BF16, tag="msg2", bufs=2)
                                nc.scalar.activation(
                                    out=msg2[:], in_=sum_t[:],
                                    func=mybir.ActivationFunctionType.Relu)
                            msgs[s4] = msg_t
                        for t in (range(clen) if do_scatter else []):
                            jh = cs + t
                            j = sub0 + jh
                            w = w_of[jh]
                            first = (t_of[jh] == 0)
                            lastt = last_of[jh]
                            if first:
                                aggr_ps = psA.tile([P, P], F32, space="PSUM",
                                                   tag="aggr")
                            m_t = mwork.tile([P, P], BF16, tag="S")
                            nc.vector.tensor_scalar(
                                out=m_t[:], in0=iota_bf[:],
                                scalar1=rel_sb[:, j:j + 1], scalar2=None,
                                op0=mybir.AluOpType.is_equal)
                            if "mask" in abl:
                                m2 = mwork.tile([P, P], BF16, tag="S2", bufs=2)
                                nc.vector.tensor_scalar(
                                    out=m2[:], in0=iota_bf[:],
                                    scalar1=rel_sb[:, j:j + 1], scalar2=None,
                                    op0=mybir.AluOpType.is_equal)
                            msg_t = msgs[(t // 4) * 4]
                            toff = (t % 4) * P
                            nc.tensor.matmul(out=aggr_ps[:],
                                             lhsT=msg_t[:, toff:toff + P],
                                             rhs=m_t[:], start=first,
                                             stop=lastt)
                            if "mm" in abl:
                                mm_d = psE.tile([P, P], F32, space="PSUM",
                                                tag="mmd")
                                nc.tensor.matmul(out=mm_d[:],
                                                 lhsT=msg_t[:, toff:toff + P],
                                                 rhs=m_t[:], start=True,
                                                 stop=True)
                            if not lastt:
                                continue
                            wp = w * P
                            if h == 0:
                                nc.scalar.copy(
                                    out=aggrA_sb[:, wp:wp + P], in_=aggr_ps[:])
                                continue
                            # finalize window w
                            t1 = work.tile([P, P], BF16, tag="t1")
                            nc.vector.tensor_add(out=t1[:], in0=aggr_ps[:],
                                                 in1=aggrA_sb[:, wp:wp + P])
                            hpre = work.tile([P, P], BF16, tag="hpre")
                            nc.vector.tensor_add(out=hpre[:], in0=t1[:],
                                                 in1=xt_sb[:, wp:wp + P])
                            mm1 = psB.tile([P, P], F32, space="PSUM", tag="mm1")
                            nc.tensor.matmul(out=mm1[:], lhsT=w1_sb[:],
                                             rhs=hpre[:], start=True, stop=True)
                            r1 = work.tile([P, P], BF16, tag="r1")
                            nc.scalar.activation(
                                out=r1[:], in_=mm1[:],
                                func=mybir.ActivationFunctionType.Relu,
                                bias=b1_sb[:, :1])
                            mm2 = psC.tile([P, P], F32, space="PSUM", tag="mm2")
                            nc.tensor.matmul(out=mm2[:], lhsT=w2_sb[:],
                                             rhs=r1[:], start=True, stop=True)
                            nc.vector.scalar_tensor_tensor(
                                out=h2_sb[:, wp:wp + P], in0=mm2[:],
                                scalar=b2_sb[:, :1], in1=xt_sb[:, wp:wp + P],
                                op0=mybir.AluOpType.add,
                                op1=mybir.AluOpType.add,
                                accum_out=sums[:, w:w + 1])
                            sqs = work.tile([P, P], F32, tag="sqs", bufs=3)
                            nc.scalar.activation(
                                out=sqs[:], in_=h2_sb[:, wp:wp + P],
                                func=mybir.ActivationFunctionType.Square,
                                accum_out=sumsq[:, w:w + 1])

            def emit_norm(alpha_ap, beta_ap):
                for w in range(nw):
                    wp = w * P
                    nrm = work.tile([P, P], F32, tag="nrm", bufs=4)
                    nc.vector.tensor_scalar(
                        out=nrm[:], in0=h2_sb[:, wp:wp + P], scalar1=alpha_ap,
                        scalar2=beta_ap, op0=mybir.AluOpType.mult,
                        op1=mybir.AluOpType.add)
                    tps = psD.tile([P, P], F32, space="PSUM", tag="tp")
                    nc.tensor.transpose(out=tps[:], in_=nrm[:],
                                        identity=ident[:])
                    ot = work.tile([P, P], F32, tag="ot", bufs=4)
                    nc.scalar.copy(out=ot[:], in_=tps[:])
                    nc.sync.dma_start(out=t_out[wp:wp + P, :], in_=ot[:])

            if repeat > 1:
                # timing mode: loop main + normalize (dummy scale/shift);
                # excludes the one-time AllGather + BN stats chain
                with tc.For_i(0, repeat, 1):
                    emit_main()
                    if not onl:
                        emit_norm(bnw_sb[:, :1], bnb_sb[:, :1])
            if not onl:
                emit_main()

            if repeat == 1 and not onl:
                # ================= BN statistics =================
                # pad-node correction: c = W2^T relu(b1) + b2
                b1bf = cpool.tile([P, 1], BF16)
                nc.scalar.activation(out=b1bf[:], in_=b1_sb[:],
                                     func=mybir.ActivationFunctionType.Relu)
                cps = psB.tile([P, 1], F32, space="PSUM", tag="mm1")
                nc.tensor.matmul(out=cps[:], lhsT=w2_sb[:], rhs=b1bf[:],
                                 start=True, stop=True)
                cvec = cpool.tile([P, 1], F32)
                nc.vector.tensor_add(out=cvec[:], in0=cps[:], in1=b2_sb[:])

                part = cpool.tile([P, 2], F32)
                nc.vector.tensor_reduce(out=part[:, 0:1], in_=sums[:],
                                        axis=mybir.AxisListType.X,
                                        op=mybir.AluOpType.add)
                nc.vector.tensor_reduce(out=part[:, 1:2], in_=sumsq[:],
                                        axis=mybir.AxisListType.X,
                                        op=mybir.AluOpType.add)
                corr = cpool.tile([P, 2], F32)
                nc.vector.tensor_mul(out=corr[:, 0:1], in0=npad_sb[:],
                                     in1=cvec[:])
                csq = cpool.tile([P, 1], F32)
                nc.vector.tensor_mul(out=csq[:], in0=cvec[:], in1=cvec[:])
                nc.vector.tensor_mul(out=corr[:, 1:2], in0=npad_sb[:],
                                     in1=csq[:])
                nc.vector.tensor_sub(out=part[:], in0=part[:], in1=corr[:])

                cin = dram.tile([P, 2], F32)
                cout = dram.tile([P, 2], F32)
                nc.sync.dma_start(out=cin[:], in_=part[:])
                nc.gpsimd.collective_compute(
                    "AllReduce", mybir.AluOpType.add,
                    replica_groups=[list(range(NCORES))],
                    ins=[cin.opt()], outs=[cout.opt()])
                stats = cpool.tile([P, 2], F32)
                nc.sync.dma_start(out=stats[:], in_=cout[:])

                inv_n = 1.0 / float(N)
                mean = cpool.tile([P, 1], F32)
                nc.vector.tensor_scalar(out=mean[:], in0=stats[:, 0:1],
                                        scalar1=inv_n, scalar2=None,
                                        op0=mybir.AluOpType.mult)
                msq = cpool.tile([P, 1], F32)
                nc.vector.tensor_scalar(out=msq[:], in0=stats[:, 1:2],
                                        scalar1=inv_n, scalar2=None,
                                        op0=mybir.AluOpType.mult)
                m2 = cpool.tile([P, 1], F32)
                nc.vector.tensor_mul(out=m2[:], in0=mean[:], in1=mean[:])
                var = cpool.tile([P, 1], F32)
                nc.vector.tensor_sub(out=var[:], in0=msq[:], in1=m2[:])
                vare = cpool.tile([P, 1], F32)
                nc.vector.tensor_scalar(out=vare[:], in0=var[:],
                                        scalar1=BN_EPS, scalar2=None,
                                        op0=mybir.AluOpType.add)
                std = cpool.tile([P, 1], F32)
                nc.scalar.activation(out=std[:], in_=vare[:],
                                     func=mybir.ActivationFunctionType.Sqrt)
                inv = cpool.tile([P, 1], F32)
                nc.vector.reciprocal(out=inv[:], in_=std[:])
                alpha = cpool.tile([P, 1], F32)
                nc.vector.tensor_mul(out=alpha[:], in0=inv[:], in1=bnw_sb[:])
                am = cpool.tile([P, 1], F32)
                nc.vector.tensor_mul(out=am[:], in0=mean[:], in1=alpha[:])
                beta = cpool.tile([P, 1], F32)
                nc.vector.tensor_sub(out=beta[:], in0=bnb_sb[:], in1=am[:])

                emit_norm(alpha[:, :1], beta[:, :1])

    nc.compile()
    return nc


# ----------------------------------------------------------------- entrypoint
_CACHE = {}


def kernel(x, edge_index, edge_attr, W1, b1, W2, b2, bn_w, bn_b):
    x = np.asarray(x, dtype=np.float32)
    edge_index = np.asarray(edge_index, dtype=np.int32)
    edge_attr = np.asarray(edge_attr, dtype=np.float32)
    N = x.shape[0]
    pp = _prep(x, edge_index, edge_attr)
    key = (N, pp["nsub"])
    if key not in _CACHE:
        _CACHE[key] = build_nc(pp["nw"], pp["tw"], pp["nsubA"], pp["nsubB"],
                               pp["npc"], pp["half"], pp["ntot"], N)
    nc = _CACHE[key]

    in_maps = make_in_maps(pp, x, W1, b1, W2, b2, bn_w, bn_b)
    res = bass_utils.run_bass_kernel_spmd(nc, in_maps,
                                          core_ids=list(range(NCORES)))
    npc = pp["npc"]
    out = np.empty((N, D), np.float32)
    for c in range(NCORES):
        lo = c * npc
        hi = min(N, lo + npc)
        out[lo:hi] = res.results[c]["out"][:hi - lo]
    return out


def make_in_maps(pp, x, W1, b1, W2, b2, bn_w, bn_b):
    in_maps = []
    for c in range(NCORES):
        in_maps.append(dict(
            xsh=pp["xsh"][c], ea=pp["ea_dev"][c], idx=pp["idx_dev"][c],
            rel=pp["rel_dev"][c],
            W1=np.asarray(W1, np.float32), W2=np.asarray(W2, np.float32),
            b1=np.asarray(b1, np.float32), b2=np.asarray(b2, np.float32),
            bn_w=np.asarray(bn_w, np.float32),
            bn_b=np.asarray(bn_b, np.float32),
            npad=pp["npad"][c],
        ))
    return in_maps


# revision 25
# speedup vs baseline: 1.1323x; 1.1323x over previous
"""GINEConv + 2-layer MLP + residual + BatchNorm on 8 Trainium2 NeuronCores.

v2 strategy (graph/data parallel, per sharding hint):
- Partition dst nodes contiguously across 8 cores (npc=6272/core). Each core
  owns the edges incident to its dst nodes.
- x is shipped as per-core bf16 shards and AllGathered on device (12.8 MB of
  H2D instead of 8x replicated fp32 x = 205 MB).
- Edge gather uses batched SWDGE dma_gather (int16 indices, one call per
  ~64 subtiles) instead of one 128-descriptor indirect DMA per subtile.
  int16 index range forces a 2-halves split of x; edges are processed in two
  passes (src < HALF, src >= HALF) with pass-A aggregates staged in SBUF.
- edge_attr is shipped fp16, host-permuted to [128, nsub, 128] so each chunk
  load is one contiguous-per-partition HWDGE DMA.
- Messages msg = relu(xg + ea) in bf16; scatter-add into a 128-node dst
  window via one-hot matmul (S built on DVE by iota/is_equal, pad edges get
  rel=-1 so their mask column is all-zero and they never land).
- Per window: h = x + aggr; h2 = x + (relu(h@W1+b1)@W2+b2) with bf16 weights
  stationary; BN partial sums accumulated on the fly (fp32).
- BN: one AllReduce of [128, 2] (sum, sumsq), biased variance, then a
  normalize + PE-transpose + store pass (fp32).

kernel(**inputs) takes FULL inputs, returns FULL [50000, 128] output.
"""
import numpy as np
import ml_dtypes

import concourse.bass as bass
import concourse.mybir as mybir
import concourse.tile as tile
import concourse.bacc as bacc
import concourse.bass_utils as bass_utils
from concourse import library_config
from concourse.masks import make_identity

P = 128
D = 128
NCORES = 8
BN_EPS = 1e-5
CH = 64          # subtiles per gather/ea chunk (64*128 = 8192 edges)

F32 = mybir.dt.float32
BF16 = mybir.dt.float16
I16 = mybir.dt.int16
BF = np.float16


# ----------------------------------------------------------------- host prep
def _prep(x, edge_index, edge_attr):
    """Partition + pad edges; build per-core arrays (identical shapes)."""
    N = x.shape[0]
    npc = ((N + NCORES - 1) // NCORES + P - 1) // P * P     # 6272
    nw = npc // P                                            # 49
    ntot = NCORES * npc                                      # 50176
    half = ((N + 1) // 2 + P - 1) // P * P                   # 25088
    assert half < 32768 and ntot - half < 32768

    src = edge_index[0].astype(np.int64)
    dst = edge_index[1].astype(np.int64)
    core = np.minimum(dst // npc, NCORES - 1)
    ldst = dst - core * npc
    win = ldst // P
    rel = ldst % P
    hlf = (src >= half).astype(np.int64)

    # counts per (core, half, window) -> shared subtile counts (max over cores)
    counts = np.zeros((NCORES, 2, nw), np.int64)
    np.add.at(counts, (core, hlf, win), 1)
    tw = np.maximum(1, (counts.max(axis=0) + P - 1) // P)    # [2, nw]
    nsubA, nsubB = int(tw[0].sum()), int(tw[1].sum())
    nsub = nsubA + nsubB
    epad = nsub * P

    # slot base for each (half, window)
    base = np.zeros((2, nw), np.int64)
    flat_tw = np.concatenate([tw[0], tw[1]])
    starts = np.concatenate([[0], (flat_tw * P).cumsum()])[:-1]
    base[0] = starts[:nw]
    base[1] = starts[nw:]

    # order edges per (core, half, window); assign slot ids
    order = np.lexsort((win, hlf, core))
    core_o, hlf_o, win_o = core[order], hlf[order], win[order]
    # inner rank within each (core, half, window) group
    grp = (core_o * 2 + hlf_o) * nw + win_o
    gcounts = np.bincount(grp, minlength=NCORES * 2 * nw)
    gstart = np.concatenate([[0], gcounts.cumsum()])[:-1]
    inner = np.arange(len(order)) - np.repeat(gstart, gcounts)
    slot = base[hlf_o, win_o] + inner

    ea_dev = np.zeros((NCORES, P, nsub, P), BF)
    idx_dev = np.zeros((NCORES, 16, nsub * 8), np.int16)
    rel_dev = np.full((NCORES, P, nsub), -1.0, np.float32)
    eab = edge_attr.astype(BF)
    for c in range(NCORES):
        m = core_o == c
        eids = order[m]
        sl = slot[m]
        s_sub, s_lane = sl // P, sl % P
        ea_dev[c, s_lane, s_sub, :] = eab[eids]
        rel_dev[c, s_lane, s_sub] = rel[eids].astype(np.float32)
        # gather indices, local to half, wrapped [i%16, i//16] per half
        lidx = (src[eids] - hlf_o[m] * half).astype(np.int16)
        gi = sl.copy()
        gi[hlf_o[m] == 1] -= nsubA * P       # position within half
        col0 = np.where(hlf_o[m] == 1, nsubA * 8, 0)
        idx_dev[c, gi % 16, col0 + gi // 16] = lidx

    # x shards (bf16, padded rows zero)
    xsh = np.zeros((NCORES, npc, D), BF)
    for c in range(NCORES):
        lo, hi = c * npc, min(N, (c + 1) * npc)
        xsh[c, :hi - lo] = x[lo:hi].astype(BF)

    npad_nodes = np.zeros((NCORES, P), np.float32)
    npad_nodes[NCORES - 1, :] = ntot - N
    return dict(nw=nw, tw=tw, nsubA=nsubA, nsubB=nsubB, nsub=nsub,
                epad=epad, npc=npc, half=half, ntot=ntot,
                ea_dev=ea_dev.reshape(NCORES, P, nsub * P),
                idx_dev=idx_dev, rel_dev=rel_dev, xsh=xsh, npad=npad_nodes)


# ------------------------------------------------------------- device program
def build_nc(nw, tw, nsubA, nsubB, npc, half, ntot, N, repeat=1, ablate="",
             only=""):
    abl = set(ablate.split(",")) if ablate else set()
    onl = set(only.split(",")) if only else set()
    nsub = nsubA + nsubB
    nc = bacc.Bacc("TRN2", target_bir_lowering=False, debug=False,
                   num_devices=NCORES)
    t_xsh = nc.dram_tensor("xsh", [npc, D], BF16, kind="ExternalInput").ap()
    t_ea = nc.dram_tensor("ea", [P, nsub * P], BF16, kind="ExternalInput").ap()
    t_idx = nc.dram_tensor("idx", [16, nsub * 8], I16, kind="ExternalInput").ap()
    t_rel = nc.dram_tensor("rel", [P, nsub], F32, kind="ExternalInput").ap()
    t_w1 = nc.dram_tensor("W1", [D, D], F32, kind="ExternalInput").ap()
    t_w2 = nc.dram_tensor("W2", [D, D], F32, kind="ExternalInput").ap()
    t_b1 = nc.dram_tensor("b1", [D], F32, kind="ExternalInput").ap()
    t_b2 = nc.dram_tensor("b2", [D], F32, kind="ExternalInput").ap()
    t_bnw = nc.dram_tensor("bn_w", [D], F32, kind="ExternalInput").ap()
    t_bnb = nc.dram_tensor("bn_b", [D], F32, kind="ExternalInput").ap()
    t_npad = nc.dram_tensor("npad", [P], F32, kind="ExternalInput").ap()
    t_out = nc.dram_tensor("out", [npc, D], F32, kind="ExternalOutput").ap()

    # per-(half, window) subtile spans
    spans = []                   # (half, w, start_subtile_in_half, count)
    for h in range(2):
        s = 0
        for w in range(nw):
            spans.append((h, w, s, int(tw[h][w])))
            s += int(tw[h][w])

    with tile.TileContext(nc) as tc:
        with (
            tc.tile_pool(name="const", bufs=1) as cpool,
            tc.tile_pool(name="iox", bufs=2) as iox,
            tc.tile_pool(name="ioe", bufs=2) as ioe,
            tc.tile_pool(name="work", bufs=10) as work,
            tc.tile_pool(name="mwork", bufs=10) as mwork,
            tc.tile_pool(name="psA", bufs=2, space="PSUM") as psA,
            tc.tile_pool(name="psB", bufs=2, space="PSUM") as psB,
            tc.tile_pool(name="psC", bufs=2, space="PSUM") as psC,
            tc.tile_pool(name="psD", bufs=2, space="PSUM") as psD,
            tc.tile_pool(name="psE", bufs=2, space="PSUM") as psE,
            tc.tile_pool(name="dram", bufs=1, space="DRAM") as dram,
        ):
            # ---- constants
            w1_sb = cpool.tile([P, D], BF16)
            nc.gpsimd.dma_start(out=w1_sb[:], in_=t_w1[:])   # cast fp32->bf16
            w2_sb = cpool.tile([P, D], BF16)
            nc.gpsimd.dma_start(out=w2_sb[:], in_=t_w2[:])
            b1_sb = cpool.tile([P, 1], F32)
            nc.sync.dma_start(out=b1_sb[:], in_=t_b1[:, None])
            b2_sb = cpool.tile([P, 1], F32)
            nc.sync.dma_start(out=b2_sb[:], in_=t_b2[:, None])
            bnw_sb = cpool.tile([P, 1], F32)
            nc.sync.dma_start(out=bnw_sb[:], in_=t_bnw[:, None])
            bnb_sb = cpool.tile([P, 1], F32)
            nc.sync.dma_start(out=bnb_sb[:], in_=t_bnb[:, None])
            npad_sb = cpool.tile([P, 1], F32)
            nc.sync.dma_start(out=npad_sb[:], in_=t_npad[:, None])
            idx_sb = cpool.tile([P, nsub * 8], I16)
            for k in range(8):
                nc.sync.dma_start(out=idx_sb[16 * k:16 * (k + 1), :],
                                  in_=t_idx[:])
            rel_sb = cpool.tile([P, nsub], F32)
            nc.sync.dma_start(out=rel_sb[:], in_=t_rel[:])
            iota_i = cpool.tile([P, P], mybir.dt.int32)
            nc.gpsimd.iota(iota_i[:], pattern=[[1, P]], base=0,
                           channel_multiplier=0)
            iota_bf = cpool.tile([P, P], BF16)
            nc.vector.tensor_copy(out=iota_bf[:], in_=iota_i[:])
            ident = cpool.tile([P, P], F32)
            make_identity(nc, ident[:])

            # xt = x_local^T via HWDGE DMA transpose (bf16)
            xt_sb = cpool.tile([P, npc], BF16)
            nc.sync.dma_start_transpose(out=xt_sb[:], in_=t_xsh[:])

            # AllGather x shards -> full x in HBM
            xin = dram.tile([npc, D], BF16)
            nc.sync.dma_start(out=xin[:], in_=t_xsh[:])
            xfull = dram.tile([ntot, D], BF16, addr_space="Shared")
            nc.gpsimd.collective_compute(
                "AllGather", mybir.AluOpType.bypass,
                replica_groups=[list(range(NCORES))],
                ins=[xin.opt()], outs=[xfull.opt()])

            aggrA_sb = cpool.tile([P, npc], BF16)
            h2_sb = cpool.tile([P, npc], F32)
            sums = cpool.tile([P, nw], F32)
            sumsq = cpool.tile([P, nw], F32)

            def emit_main():
                for h in range(2):
                    nsub_h = nsubA if h == 0 else nsubB
                    col0 = 0 if h == 0 else nsubA * 8
                    sub0 = 0 if h == 0 else nsubA
                    src_lo = 0 if h == 0 else half
                    src_hi = half if h == 0 else ntot
                    # window lookup for this half
                    w_of, t_of, last_of = [], [], []
                    for (hh, w, s, cnt) in spans:
                        if hh != h:
                            continue
                        for t in range(cnt):
                            w_of.append(w)
                            t_of.append(t)
                            last_of.append(t == cnt - 1)
                    aggr_ps = None
                    for cs in range(0, nsub_h, CH):
                        clen = min(CH, nsub_h - cs)
                        xg = iox.tile([P, clen * P], BF16, tag="xg")
                        nc.gpsimd.dma_gather(
                            xg[:].rearrange("p (s f) -> p s f", f=P),
                            xfull[src_lo:src_hi, :],
                            idx_sb[:, col0 + cs * 8: col0 + (cs + clen) * 8],
                            clen * P, clen * P, P, single_packet=False)
                        if "gather" in abl:
                            xg2 = iox.tile([P, clen * P], BF16, tag="xg2", bufs=1)
                            nc.gpsimd.dma_gather(
                                xg2[:].rearrange("p (s f) -> p s f", f=P),
                                xfull[src_lo:src_hi, :],
                                idx_sb[:, col0 + cs * 8:
                                       col0 + (cs + clen) * 8],
                                clen * P, clen * P, P, single_packet=False)
                        ea_t = None
                        if not onl or "ea" in onl:
                            ea_t = ioe.tile([P, clen * P], BF16, tag="ea")
                            nc.sync.dma_start(
                                out=ea_t[:],
                                in_=t_ea[:, (sub0 + cs) * P:
                                         (sub0 + cs + clen) * P])
                        if "ea" in abl:
                            ea2 = ioe.tile([P, clen * P], BF16, tag="ea2", bufs=1)
                            nc.sync.dma_start(
                                out=ea2[:],
                                in_=t_ea[:, (sub0 + cs) * P:
                                         (sub0 + cs + clen) * P])
                        msgs = {}
                        do_msg = (not onl) or "msg" in onl
                        do_scatter = not onl
                        for s4 in (range(0, clen, 4) if do_msg else []):
                            sl = min(4, clen - s4)
                            sum_t = work.tile([P, sl * P], BF16, tag="sum", bufs=4)
                            nc.vector.tensor_add(
                                out=sum_t[:], in0=xg[:, s4 * P:(s4 + sl) * P],
                                in1=ea_t[:, s4 * P:(s4 + sl) * P])
                            if "add" in abl:
                                sum2 = work.tile([P, sl * P], BF16, tag="sum2", bufs=2)
                                nc.vector.tensor_add(
                                    out=sum2[:],
                                    in0=xg[:, s4 * P:(s4 + sl) * P],
                                    in1=ea_t[:, s4 * P:(s4 + sl) * P])
                            msg_t = work.tile([P, sl * P], BF16, tag="msg", bufs=6)
                            nc.scalar.activation(
                                out=msg_t[:], in_=sum_t[:],
                                func=mybir.ActivationFunctionType.Relu)
                            if "relu" in abl:
                                msg2 = work.tile([P, sl * P], BF16, tag="msg2", bufs=2)
                                nc.scalar.activation(
                                    out=msg2[:], in_=sum_t[:],
                                    func=mybir.ActivationFunctionType.Relu)
                            msgs[s4] = msg_t
                        for t in (range(clen) if do_scatter else []):
                            jh = cs + t
                            j = sub0 + jh
                            w = w_of[jh]
                            first = (t_of[jh] == 0)
                            lastt = last_of[jh]
                            if first:
                                aggr_ps = psA.tile([P, P], F32, space="PSUM",
                                                   tag="aggr")
                            m_t = mwork.tile([P, P], BF16, tag="S")
                            nc.vector.tensor_scalar(
                                out=m_t[:], in0=iota_bf[:],
                                scalar1=rel_sb[:, j:j + 1], scalar2=None,
                                op0=mybir.AluOpType.is_equal)
                            if "mask" in abl:
                                m2 = mwork.tile([P, P], BF16, tag="S2", bufs=2)
                                nc.vector.tensor_scalar(
                                    out=m2[:], in0=iota_bf[:],
                                    scalar1=rel_sb[:, j:j + 1], scalar2=None,
                                    op0=mybir.AluOpType.is_equal)
                            msg_t = msgs[(t // 4) * 4]
                            toff = (t % 4) * P
                            nc.tensor.matmul(out=aggr_ps[:],
                                             lhsT=msg_t[:, toff:toff + P],
                                             rhs=m_t[:], start=first,
                                             stop=lastt)
                            if "mm" in abl:
                                mm_d = psE.tile([P, P], F32, space="PSUM",
                                                tag="mmd")
                                nc.tensor.matmul(out=mm_d[:],
                                                 lhsT=msg_t[:, toff:toff + P],
                                                 rhs=m_t[:], start=True,
                                                 stop=True)
                            if not lastt:
                                continue
                            wp = w * P
                            if h == 0:
                                nc.scalar.copy(
                                    out=aggrA_sb[:, wp:wp + P], in_=aggr_ps[:])
                                continue
                            # finalize window w
                            t1 = work.tile([P, P], BF16, tag="t1")
                            nc.vector.tensor_add(out=t1[:], in0=aggr_ps[:],
                                                 in1=aggrA_sb[:, wp:wp + P])
                            hpre = work.tile([P, P], BF16, tag="hpre")
                            nc.vector.tensor_add(out=hpre[:], in0=t1[:],
                                                 in1=xt_sb[:, wp:wp + P])
                            mm1 = psB.tile([P, P], F32, space="PSUM", tag="mm1")
                            nc.tensor.matmul(out=mm1[:], lhsT=w1_sb[:],
                                             rhs=hpre[:], start=True, stop=True)
                            r1 = work.tile([P, P], BF16, tag="r1")
                            nc.scalar.activation(
                                out=r1[:], in_=mm1[:],
                                func=mybir.ActivationFunctionType.Relu,
                                bias=b1_sb[:, :1])
                            mm2 = psC.tile([P, P], F32, space="PSUM", tag="mm2")
                            nc.tensor.matmul(out=mm2[:], lhsT=w2_sb[:],
                                             rhs=r1[:], start=True, stop=True)
                            nc.vector.scalar_tensor_tensor(
                                out=h2_sb[:, wp:wp + P], in0=mm2[:],
                                scalar=b2_sb[:, :1], in1=xt_sb[:, wp:wp + P],
                                op0=mybir.AluOpType.add,
                                op1=mybir.AluOpType.add,
                                accum_out=sums[:, w:w + 1])
                            sqs = work.tile([P, P], F32, tag="sqs", bufs=3)
                            nc.scalar.activation(
                                out=sqs[:], in_=h2_sb[:, wp:wp + P],
                                func=mybir.ActivationFunctionType.Square,
                                accum_out=sumsq[:, w:w + 1])

            def emit_norm(alpha_ap, beta_ap):
                for w in range(nw):
                    wp = w * P
                    nrm = work.tile([P, P], F32, tag="nrm", bufs=4)
                    nc.vector.tensor_scalar(
                        out=nrm[:], in0=h2_sb[:, wp:wp + P], scalar1=alpha_ap,
                        scalar2=beta_ap, op0=mybir.AluOpType.mult,
                        op1=mybir.AluOpType.add)
                    tps = psD.tile([P, P], F32, space="PSUM", tag="tp")
                    nc.tensor.transpose(out=tps[:], in_=nrm[:],
                                        identity=ident[:])
                    ot = work.tile([P, P], F32, tag="ot", bufs=4)
                    nc.scalar.copy(out=ot[:], in_=tps[:])
                    nc.sync.dma_start(out=t_out[wp:wp + P, :], in_=ot[:])

            if repeat > 1:
                # timing mode: loop main + normalize (dummy scale/shift);
                # excludes the one-time AllGather + BN stats chain
                with tc.For_i(0, repeat, 1):
                    emit_main()
                    if not onl:
                        emit_norm(bnw_sb[:, :1], bnb_sb[:, :1])
            if not onl:
                emit_main()

            if repeat == 1 and not onl:
                # ================= BN statistics =================
                # pad-node correction: c = W2^T relu(b1) + b2
                b1bf = cpool.tile([P, 1], BF16)
                nc.scalar.activation(out=b1bf[:], in_=b1_sb[:],
                                     func=mybir.ActivationFunctionType.Relu)
                cps = psB.tile([P, 1], F32, space="PSUM", tag="mm1")
                nc.tensor.matmul(out=cps[:], lhsT=w2_sb[:], rhs=b1bf[:],
                                 start=True, stop=True)
                cvec = cpool.tile([P, 1], F32)
                nc.vector.tensor_add(out=cvec[:], in0=cps[:], in1=b2_sb[:])

                part = cpool.tile([P, 2], F32)
                nc.vector.tensor_reduce(out=part[:, 0:1], in_=sums[:],
                                        axis=mybir.AxisListType.X,
                                        op=mybir.AluOpType.add)
                nc.vector.tensor_reduce(out=part[:, 1:2], in_=sumsq[:],
                                        axis=mybir.AxisListType.X,
                                        op=mybir.AluOpType.add)
                corr = cpool.tile([P, 2], F32)
                nc.vector.tensor_mul(out=corr[:, 0:1], in0=npad_sb[:],
                                     in1=cvec[:])
                csq = cpool.tile([P, 1], F32)
                nc.vector.tensor_mul(out=csq[:], in0=cvec[:], in1=cvec[:])
                nc.vector.tensor_mul(out=corr[:, 1:2], in0=npad_sb[:],
                                     in1=csq[:])
                nc.vector.tensor_sub(out=part[:], in0=part[:], in1=corr[:])

                cin = dram.tile([P, 2], F32)
                cout = dram.tile([P, 2], F32)
                nc.sync.dma_start(out=cin[:], in_=part[:])
                nc.gpsimd.collective_compute(
                    "AllReduce", mybir.AluOpType.add,
                    replica_groups=[list(range(NCORES))],
                    ins=[cin.opt()], outs=[cout.opt()])
                stats = cpool.tile([P, 2], F32)
                nc.sync.dma_start(out=stats[:], in_=cout[:])

                inv_n = 1.0 / float(N)
                mean = cpool.tile([P, 1], F32)
                nc.vector.tensor_scalar(out=mean[:], in0=stats[:, 0:1],
                                        scalar1=inv_n, scalar2=None,
                                        op0=mybir.AluOpType.mult)
                msq = cpool.tile([P, 1], F32)
                nc.vector.tensor_scalar(out=msq[:], in0=stats[:, 1:2],
                                        scalar1=inv_n, scalar2=None,
                                        op0=mybir.AluOpType.mult)
                m2 = cpool.tile([P, 1], F32)
                nc.vector.tensor_mul(out=m2[:], in0=mean[:], in1=mean[:])
                var = cpool.tile([P, 1], F32)
                nc.vector.tensor_sub(out=var[:], in0=msq[:], in1=m2[:])
                vare = cpool.tile([P, 1], F32)
                nc.vector.tensor_scalar(out=vare[:], in0=var[:],
                                        scalar1=BN_EPS, scalar2=None,
                                        op0=mybir.AluOpType.add)
                std = cpool.tile([P, 1], F32)
                nc.scalar.activation(out=std[:], in_=vare[:],
                                     func=mybir.ActivationFunctionType.Sqrt)
                inv = cpool.tile([P, 1], F32)
                nc.vector.reciprocal(out=inv[:], in_=std[:])
                alpha = cpool.tile([P, 1], F32)
                nc.vector.tensor_mul(out=alpha[:], in0=inv[:], in1=bnw_sb[:])
                am = cpool.tile([P, 1], F32)
                nc.vector.tensor_mul(out=am[:], in0=mean[:], in1=alpha[:])
                beta = cpool.tile([P, 1], F32)
                nc.vector.tensor_sub(out=beta[:], in0=bnb_sb[:], in1=am[:])

                emit_norm(alpha[:, :1], beta[:, :1])

    nc.compile()
    return nc


# ----------------------------------------------------------------- entrypoint
_CACHE = {}


def kernel(x, edge_index, edge_attr, W1, b1, W2, b2, bn_w, bn_b):
    x = np.asarray(x, dtype=np.float32)
    edge_index = np.asarray(edge_index, dtype=np.int32)
    edge_attr = np.asarray(edge_attr, dtype=np.float32)
    N = x.shape[0]
    pp = _prep(x, edge_index, edge_attr)
    key = (N, pp["nsub"])
    if key not in _CACHE:
        _CACHE[key] = build_nc(pp["nw"], pp["tw"], pp["nsubA"], pp["nsubB"],
                               pp["npc"], pp["half"], pp["ntot"], N)
    nc = _CACHE[key]

    in_maps = make_in_maps(pp, x, W1, b1, W2, b2, bn_w, bn_b)
    res = bass_utils.run_bass_kernel_spmd(nc, in_maps,
                                          core_ids=list(range(NCORES)))
    npc = pp["npc"]
    out = np.empty((N, D), np.float32)
    for c in range(NCORES):
        lo = c * npc
        hi = min(N, lo + npc)
        out[lo:hi] = res.results[c]["out"][:hi - lo]
    return out


def make_in_maps(pp, x, W1, b1, W2, b2, bn_w, bn_b):
    in_maps = []
    for c in range(NCORES):
        in_maps.append(dict(
            xsh=pp["xsh"][c], ea=pp["ea_dev"][c], idx=pp["idx_dev"][c],
            rel=pp["rel_dev"][c],
            W1=np.asarray(W1, np.float32), W2=np.asarray(W2, np.float32),
            b1=np.asarray(b1, np.float32), b2=np.asarray(b2, np.float32),
            bn_w=np.asarray(bn_w, np.float32),
            bn_b=np.asarray(bn_b, np.float32),
            npad=pp["npad"][c],
        ))
    return in_maps
